# revision 14
# baseline (speedup 1.0000x reference)
# Trainium2 Bass kernel for nn_DependencyParser (2-layer biLSTM + edge-grid MLP).
#
# Strategy (8 NeuronCores):
#  - The n x n score grid is sharded row-wise: core c owns head rows [96c, 96c+96).
#  - The sequential biLSTM is time-chunked: core c computes LSTM states for its
#    own 96-step window after a 48-step warmup from zero state (the LSTM state
#    is contractive, so the warmup converges to the exact trajectory to ~1e-7).
#    Cores 0/7 have no real warmup; their fictitious warmup steps are "frozen"
#    by adding -40 to the i/o gate pre-activations so h,c stay ~0 (exact to fp32).
#  - After each layer, the owned windows are AllGathered so every core has the
#    full sequence for the next layer's input transform / the grid MLP.
#  - Recurrent matvec on the PE (bf16 weights); gates/states fp32; grid MLP in
#    float32r (full-rate fp32 matmul mode). The last matvec (w3) uses h2 as the
#    stationary operand so scores come out j-major (host un-transposes).

import numpy as np
import ml_dtypes

import concourse.bacc as bacc
import concourse.bass as bass
import concourse.mybir as mybir
import concourse.tile as tile
from concourse.bass import ds
from concourse.bass_utils import run_bass_kernel_spmd

AF = mybir.ActivationFunctionType
ALU = mybir.AluOpType
AX = mybir.AxisListType
F32 = mybir.dt.float32
F32R = mybir.dt.float32r
BF16 = mybir.dt.bfloat16

N = 768          # sequence length / grid size
EMB = 200        # input embedding dim
H = 256          # LSTM hidden per direction
NCORES = 8
OWN = N // NCORES   # 96 owned time steps / head rows per core
WARM = 32           # warmup steps for chunked recurrence
CHL = OWN + WARM    # chain length per core per direction (128)
FREEZE = -40.0      # gate preactivation offset that freezes the LSTM state
JB = N // 128       # 6 j-blocks of 128
NT = 2
JT = N // NT        # 384

# device gate layout: 8 col-blocks of 128 = [i0 i1 f0 f1 o0 o1 g0 g1]
_PERM_BLOCKS = [0, 1, 2, 3, 6, 7, 4, 5]   # source block (pytorch i,f,g,o order)
_FREEZE_M = (0, 1, 4, 5)                  # i and o col-blocks get FREEZE in warmup
PERM = np.concatenate([np.arange(b * 128, (b + 1) * 128) for b in _PERM_BLOCKS])


# ----------------------------------------------------------- host weight prep

def _whhT(whh):
    """[4H, H] -> [128, 2*8*128] bf16 PE lhsT blocks."""
    wp = np.asarray(whh, np.float32)[PERM]
    out = np.zeros((128, 2, 8, 128), np.float32)
    for kc in range(2):
        for m in range(8):
            out[:, kc, m, :] = wp[m * 128:(m + 1) * 128, kc * 128:(kc + 1) * 128].T
    return out.reshape(128, -1).astype(ml_dtypes.bfloat16)


def _wihT(wih, bih, bhh, din, nkc):
    """[4H, din] -> [128, nkc*8*128] fp32 lhsT; bias row at din, freeze at din+1."""
    wp = np.asarray(wih, np.float32)[PERM]
    bp = (np.asarray(bih, np.float32) + np.asarray(bhh, np.float32))[PERM]
    aug = np.zeros((1024, nkc * 128), np.float32)
    aug[:, :din] = wp
    aug[:, din] = bp
    for m in _FREEZE_M:
        aug[m * 128:(m + 1) * 128, din + 1] = FREEZE
    out = np.zeros((128, nkc, 8, 128), np.float32)
    for kc in range(nkc):
        for m in range(8):
            out[:, kc, m, :] = aug[m * 128:(m + 1) * 128, kc * 128:(kc + 1) * 128].T
    return out.reshape(128, -1)


def _x_windows(x, core):
    """Per-core augmented input windows, fp32 [128, 2, CHL] for f and b dirs."""
    x = np.asarray(x, np.float32)
    outs = []
    for t0 in (core * OWN - WARM, core * OWN):
        w = np.zeros((256, CHL), np.float32)
        for tt in range(CHL):
            t = t0 + tt
            if 0 <= t < N:
                w[:EMB, tt] = x[t]
                w[EMB, tt] = 1.0
            else:
                w[EMB, tt] = 1.0
                w[EMB + 1, tt] = 1.0
        outs.append(w.reshape(2, 128, CHL).transpose(1, 0, 2).copy())
    return outs  # [xf, xb]


def _w1T(w1half, b1=None):
    """[256, 512] -> [128, 5*2*128] fp32 lhsT blocks (+bias chunk rows if b1)."""
    w = np.asarray(w1half, np.float32)
    out = np.zeros((128, 5, 2, 128), np.float32)
    for kc in range(4):
        for m in range(2):
            out[:, kc, m, :] = w[m * 128:(m + 1) * 128, kc * 128:(kc + 1) * 128].T
    if b1 is not None:
        b = np.asarray(b1, np.float32)
        for m in range(2):
            out[0, 4, m, :] = b[m * 128:(m + 1) * 128]
    return out.reshape(128, -1)


def _freeze_row(core, lo_frozen_core, hi_frozen_core):
    """[1, CHL] warmup-freeze indicator rows for phase-B aug chunk."""
    r = np.zeros((1, CHL), np.float32)
    if core == lo_frozen_core:
        r[0, :WARM] = 1.0
    if core == hi_frozen_core:
        r[0, CHL - WARM:] = 1.0
    return r


# --------------------------------------------------------------- bass program

def build(debug=False):
    nc = bacc.Bacc("TRN2", target_bir_lowering=False, debug=False,
                   num_devices=NCORES)

    def din(name, shape, dt=F32):
        return nc.dram_tensor(name, shape, dt, kind="ExternalInput")

    i_xf = din("xf", [128, 2, CHL])
    i_xb = din("xb", [128, 2, CHL])
    i_wih0f = din("wih0f", [128, 2 * 8 * 128])
    i_wih0b = din("wih0b", [128, 2 * 8 * 128])
    i_wih1f = din("wih1f", [128, 5 * 8 * 128])
    i_wih1b = din("wih1b", [128, 5 * 8 * 128])
    i_whh = {d: din(f"whh{d}", [128, 2 * 8 * 128], BF16)
             for d in ("0f", "0b", "1f", "1b")}
    i_frzf = din("frzf", [1, CHL])
    i_frzb = din("frzb", [1, CHL])
    i_w1h = din("w1h", [128, 5 * 2 * 128], F32R)
    i_w1c = din("w1c", [128, 4 * 2 * 128], F32R)
    i_w2 = din("w2", [128, 2 * 128], F32R)
    i_w3 = din("w3", [128, 1], BF16)
    i_b2 = din("b2", [128, 1])
    i_b3 = din("b3", [128, 1])
    i_mask = din("maskT", [128, JB, OWN])

    o_score = nc.dram_tensor("scoreT", [128, JB * OWN], F32, kind="ExternalOutput")
    o_colsum = nc.dram_tensor("colsumT", [128, JB], F32, kind="ExternalOutput")
    dbg = {}
    if debug:
        for nm, shp, dt in (("d_xw0f", [128, 8 * CHL], F32),
                            ("d_h0", [128, 4 * (N + 2 * WARM)], F32),
                            ("d_xw1f", [128, 8 * CHL], F32),
                            ("d_emb", [128, 4 * N], F32R),
                            ("d_aT", [128, 2 * OWN], F32),
                            ("d_cT", [128, 2 * N], F32)):
            dbg[nm] = nc.dram_tensor(nm, shp, dt, kind="ExternalOutput")

    HPAD = N + 2 * WARM

    with tile.TileContext(nc) as tc:
        with (
            tc.tile_pool(name="persist", bufs=1) as P,
            tc.tile_pool(name="mmp", bufs=2, space="PSUM") as MMP,
            tc.tile_pool(name="dram", bufs=1, space="DRAM") as DR,
        ):
            # ---- load static inputs to SBUF
            xf = P.tile([128, 2, CHL], F32)
            nc.sync.dma_start(xf[:], i_xf[:])
            xb = P.tile([128, 2, CHL], F32)
            nc.sync.dma_start(xb[:], i_xb[:])
            wih0 = {}
            for d, inp in (("f", i_wih0f), ("b", i_wih0b)):
                t = P.tile([128, 2, 8, 128], F32, tag=f"wih0{d}")
                nc.sync.dma_start(t[:], inp[:].rearrange("p (a b c) -> p a b c", a=2, b=8))
                wih0[d] = t
            wih1 = {}
            for d, inp in (("f", i_wih1f), ("b", i_wih1b)):
                t = P.tile([128, 5, 8, 128], F32, tag=f"wih1{d}")
                nc.sync.dma_start(t[:], inp[:].rearrange("p (a b c) -> p a b c", a=5, b=8))
                wih1[d] = t
            whh = {}
            for d in ("0f", "0b", "1f", "1b"):
                t = P.tile([128, 2, 8, 128], BF16, tag=f"whh{d}")
                nc.sync.dma_start(t[:], i_whh[d][:].rearrange("p (a b c) -> p a b c", a=2, b=8))
                whh[d] = t
            w1h = P.tile([128, 5, 2, 128], F32R)
            nc.sync.dma_start(w1h[:], i_w1h[:].rearrange("p (a b c) -> p a b c", a=5, b=2))
            w1c = P.tile([128, 4, 2, 128], F32R)
            nc.sync.dma_start(w1c[:], i_w1c[:].rearrange("p (a b c) -> p a b c", a=4, b=2))
            w2 = P.tile([128, 2, 128], F32R)
            nc.sync.dma_start(w2[:], i_w2[:].rearrange("p (a b) -> p a b", a=2))
            w3 = P.tile([128, 1], BF16)
            nc.sync.dma_start(w3[:], i_w3[:])
            b2 = P.tile([128, 1], F32)
            nc.sync.dma_start(b2[:], i_b2[:])
            b3 = P.tile([128, 1], F32)
            nc.sync.dma_start(b3[:], i_b3[:])
            maskT = P.tile([128, JB, OWN], F32)
            nc.sync.dma_start(maskT[:], i_mask[:])

            pid = nc.vector.partition_id()

            def xw_from(lhsT, rhs, nkc, xw_out):
                """xw_out[128, 8, CHL] fp32 = sum_kc lhsT[:,kc,m,:].T @ rhs[:,kc,:]"""
                for m in range(8):
                    ps = MMP.tile([128, CHL], F32, tag="mmp")
                    for kc in range(nkc):
                        nc.tensor.matmul(ps[:], lhsT[:, kc, m, :], rhs[:, kc, :],
                                         start=(kc == 0), stop=(kc == nkc - 1))
                    nc.vector.tensor_copy(xw_out[:, m, :], ps[:])

            xw_f = P.tile([128, 8, CHL], F32)
            xw_b = P.tile([128, 8, CHL], F32)
            h_ownT = P.tile([128, 4, OWN], F32)
            emb_ownT = P.tile([128, 4, OWN], F32R)
            h0_pad = P.tile([128, 4, HPAD], F32)

            with (
                tc.tile_pool(name="small", bufs=3) as SM,
                tc.tile_pool(name="hp", bufs=2) as HP,
                tc.tile_pool(name="gpsf", bufs=2, space="PSUM") as GPSF,
                tc.tile_pool(name="gpsb", bufs=2, space="PSUM") as GPSB,
            ):
                def chain(name, whhT, xw_sb, gpool, gtag, storeT, store_base, rev):
                    """One LSTM chain of CHL steps; owned h written to storeT."""
                    TC = P.tile([128, 4], F32, tag=f"TC_{name}")  # [tg0 tg1 c0 c1]
                    nc.vector.memset(TC[:, 2:4], 0.0)
                    h_bf = HP.tile([128, 2], BF16, tag=f"h_{name}")
                    nc.vector.memset(h_bf[:], 0.0)
                    for ss in range(CHL):
                        col = (CHL - 1 - ss) if rev else ss
                        gps = gpool.tile([128, 8], F32, tag=gtag)
                        for m in range(8):
                            nc.tensor.matmul(gps[:, m:m + 1], whhT[:, 0, m, :],
                                             h_bf[:, 0:1], start=True, stop=False)
                            nc.tensor.matmul(gps[:, m:m + 1], whhT[:, 1, m, :],
                                             h_bf[:, 1:2], start=False, stop=True)
                        gates = SM.tile([128, 8], F32, tag=f"g_{name}")
                        nc.vector.tensor_tensor(gates[:], gps[:], xw_sb[:, :, col],
                                                ALU.add)
                        S = SM.tile([128, 6], F32, tag=f"S_{name}")
                        nc.scalar.activation(S[:], gates[:, 0:6], AF.Sigmoid)
                        nc.scalar.activation(TC[:, 0:2], gates[:, 6:8], AF.Tanh)
                        Pp = SM.tile([128, 4], F32, tag=f"P_{name}")
                        nc.vector.tensor_tensor(Pp[:], S[:, 0:4], TC[:], ALU.mult)
                        nc.vector.tensor_tensor(TC[:, 2:4], Pp[:, 0:2], Pp[:, 2:4],
                                                ALU.add)
                        T2 = SM.tile([128, 2], F32, tag=f"T2_{name}")
                        nc.scalar.activation(T2[:], TC[:, 2:4], AF.Tanh)
                        h_bf = HP.tile([128, 2], BF16, tag=f"h_{name}")
                        nc.vector.tensor_tensor(h_bf[:], S[:, 4:6], T2[:], ALU.mult)
                        if ss >= WARM:
                            idx = (ss - WARM) if not rev else (CHL - 1 - ss)
                            nc.vector.tensor_tensor(
                                storeT[:, store_base:store_base + 2, idx:idx + 1],
                                S[:, 4:6], T2[:], ALU.mult)

                # ========== phase A: layer 0 ==========
                xw_from(wih0["f"], xf, 2, xw_f)
                xw_from(wih0["b"], xb, 2, xw_b)
                if debug:
                    nc.sync.dma_start(dbg["d_xw0f"][:],
                                      xw_f[:].rearrange("p a b -> p (a b)"))

                chain("a_f", whh["0f"], xw_f, GPSF, "gf", h_ownT, 0, rev=False)
                chain("a_b", whh["0b"], xw_b, GPSB, "gb", h_ownT, 2, rev=True)

                cc1_in = DR.tile([4, 128, OWN], F32)
                cc1_out = DR.tile([NCORES, 4, 128, OWN], F32, addr_space="Shared")
                nc.sync.dma_start(cc1_in[:].rearrange("c p k -> p c k"), h_ownT[:])
                nc.gpsimd.collective_compute(
                    "AllGather", ALU.bypass,
                    replica_groups=[list(range(NCORES))],
                    ins=[cc1_in.opt()], outs=[cc1_out.opt()])

                nc.vector.memset(h0_pad[:, :, 0:WARM], 0.0)
                nc.vector.memset(h0_pad[:, :, WARM + N:], 0.0)
                for ch in range(4):
                    nc.sync.dma_start(
                        h0_pad[:, ch, WARM:WARM + N].rearrange("p (b k) -> p b k", k=OWN),
                        cc1_out[:, ch, :, :].rearrange("b p k -> p b k"))
                if debug:
                    nc.sync.dma_start(dbg["d_h0"][:],
                                      h0_pad[:].rearrange("p a b -> p (a b)"))

                # ========== phase B: layer 1 ==========
                h0w = {}
                for d, off, frz in (("f", pid * OWN, i_frzf),
                                    ("b", pid * OWN + WARM, i_frzb)):
                    hw = P.tile([128, 5, CHL], F32, tag=f"h0w_{d}")
                    nc.vector.tensor_copy(hw[:, 0:4, :], h0_pad[:, :, ds(off, CHL)])
                    nc.vector.memset(hw[:, 4, :], 0.0)
                    nc.vector.memset(hw[0:1, 4, :], 1.0)
                    nc.sync.dma_start(hw[1:2, 4, :], frz[:])
                    h0w[d] = hw

                xw_from(wih1["f"], h0w["f"], 5, xw_f)
                xw_from(wih1["b"], h0w["b"], 5, xw_b)
                if debug:
                    nc.sync.dma_start(dbg["d_xw1f"][:],
                                      xw_f[:].rearrange("p a b -> p (a b)"))

                chain("b_f", whh["1f"], xw_f, GPSF, "gf", emb_ownT, 0, rev=False)
                chain("b_b", whh["1b"], xw_b, GPSB, "gb", emb_ownT, 2, rev=True)

            # ---- AllGather embeddings
            cc2_in = DR.tile([4, 128, OWN], F32R)
            cc2_out = DR.tile([NCORES, 4, 128, OWN], F32R, addr_space="Shared")
            nc.sync.dma_start(cc2_in[:].rearrange("c p k -> p c k"), emb_ownT[:])
            nc.gpsimd.collective_compute(
                "AllGather", ALU.bypass,
                replica_groups=[list(range(NCORES))],
                ins=[cc2_in.opt()], outs=[cc2_out.opt()])

            embT = P.tile([128, 4, N], F32R)
            for ch in range(4):
                nc.sync.dma_start(
                    embT[:, ch, :].rearrange("p (b k) -> p b k", k=OWN),
                    cc2_out[:, ch, :, :].rearrange("b p k -> p b k"))
            if debug:
                nc.sync.dma_start(dbg["d_emb"][:],
                                  embT[:].rearrange("p a b -> p (a b)"))

            # ========== grid MLP (scores come out j-major) ==========
            with (
                tc.tile_pool(name="gridsm", bufs=3) as GSM,
                tc.tile_pool(name="rawp", bufs=1, space="PSUM") as RAWP,
            ):
                ones1f = P.tile([1, OWN], F32)
                nc.vector.memset(ones1f[:], 1.0)
                ones1 = P.tile([1, OWN], F32R)
                nc.vector.tensor_copy(ones1[:], ones1f[:])

                aT = P.tile([128, 2, OWN], F32)
                for m in range(2):
                    ps = MMP.tile([128, OWN], F32, tag="mmp")
                    for kc in range(4):
                        nc.tensor.matmul(ps[:], w1h[:, kc, m, :], emb_ownT[:, kc, :],
                                         start=(kc == 0), stop=False)
                    nc.tensor.matmul(ps[:], w1h[0:1, 4, m, :], ones1[:],
                                     start=False, stop=True)
                    nc.vector.tensor_copy(aT[:, m, :], ps[:])
                if debug:
                    nc.sync.dma_start(dbg["d_aT"][:],
                                      aT[:].rearrange("p a b -> p (a b)"))

                cT = P.tile([128, 2, N], F32)
                for m in range(2):
                    for nt in range(NT):
                        ps = MMP.tile([128, JT], F32, tag="mmp")
                        for kc in range(4):
                            nc.tensor.matmul(ps[:], w1c[:, kc, m, :],
                                             embT[:, kc, nt * JT:(nt + 1) * JT],
                                             start=(kc == 0), stop=(kc == 3))
                        nc.vector.tensor_copy(cT[:, m, nt * JT:(nt + 1) * JT], ps[:])
                if debug:
                    nc.sync.dma_start(dbg["d_cT"][:],
                                      cT[:].rearrange("p a b -> p (a b)"))

                rawT = RAWP.tile([128, JB, OWN], F32)
                for i in range(OWN):
                    for nt in range(NT):
                        h1 = GSM.tile([128, 2, JT], F32R, tag="h1")
                        for m in range(2):
                            nc.vector.tensor_scalar(
                                h1[:, m, :], cT[:, m, nt * JT:(nt + 1) * JT],
                                aT[:, m, i:i + 1], 0.0, op0=ALU.add, op1=ALU.max)
                        h2ps = MMP.tile([128, JT], F32, tag="mmp")
                        nc.tensor.matmul(h2ps[:], w2[:, 0, :], h1[:, 0, :],
                                         start=True, stop=False)
                        nc.tensor.matmul(h2ps[:], w2[:, 1, :], h1[:, 1, :],
                                         start=False, stop=True)
                        h2r = GSM.tile([128, JT], BF16, tag="h2r")
                        nc.scalar.activation(h2r[:], h2ps[:], AF.Relu, bias=b2[:, 0:1])
                        for jb in range(3):
                            g = nt * 3 + jb
                            nc.tensor.matmul(rawT[:, g, i:i + 1],
                                             h2r[:, jb * 128:(jb + 1) * 128], w3[:],
                                             start=True, stop=True)

                scoreT = P.tile([128, JB, OWN], F32)
                nc.scalar.activation(scoreT[:].rearrange("p a b -> p (a b)"),
                                     rawT[:].rearrange("p a b -> p (a b)"),
                                     AF.Relu, bias=b3[:, 0:1])
                scoreM = P.tile([128, JB, OWN], F32)
                nc.vector.tensor_tensor(scoreM[:], scoreT[:], maskT[:], ALU.mult)
                nc.sync.dma_start(o_score[:],
                                  scoreM[:].rearrange("p a b -> p (a b)"))

                esT = P.tile([128, JB, OWN], F32)
                nc.scalar.activation(esT[:].rearrange("p a b -> p (a b)"),
                                     scoreM[:].rearrange("p a b -> p (a b)"), AF.Exp)
                csT = P.tile([128, JB], F32)
                for g in range(JB):
                    nc.vector.tensor_reduce(csT[:, g:g + 1], esT[:, g, :],
                                            axis=AX.X, op=ALU.add)
                nc.sync.dma_start(o_colsum[:], csT[:])

    nc.compile()
    return nc


# ------------------------------------------------------------------ host glue

_NC_CACHE = {}


def _get_nc(debug=False):
    key = bool(debug)
    if key not in _NC_CACHE:
        _NC_CACHE[key] = build(debug)
    return _NC_CACHE[key]


def _in_maps(inputs):
    inp = {k: np.asarray(v) for k, v in inputs.items()}
    x = inp["x"].astype(np.float32)

    shared = {
        "wih0f": _wihT(inp["wih_l0f"], inp["bih_l0f"], inp["bhh_l0f"], EMB, 2),
        "wih0b": _wihT(inp["wih_l0b"], inp["bih_l0b"], inp["bhh_l0b"], EMB, 2),
        "wih1f": _wihT(inp["wih_l1f"], inp["bih_l1f"], inp["bhh_l1f"], 2 * H, 5),
        "wih1b": _wihT(inp["wih_l1b"], inp["bih_l1b"], inp["bhh_l1b"], 2 * H, 5),
        "whh0f": _whhT(inp["whh_l0f"]), "whh0b": _whhT(inp["whh_l0b"]),
        "whh1f": _whhT(inp["whh_l1f"]), "whh1b": _whhT(inp["whh_l1b"]),
        "w1h": _w1T(inp["w1"][:, :2 * H], inp["b1"]),
        "w1c": _w1T(inp["w1"][:, 2 * H:])[:, :4 * 2 * 128],
        "w2": np.ascontiguousarray(
            np.asarray(inp["w2"], np.float32).T.reshape(2, 128, 128)
            .transpose(1, 0, 2)).reshape(128, 256),
        "w3": np.asarray(inp["w3"], np.float32).reshape(128, 1).astype(
            ml_dtypes.bfloat16),
        "b2": np.asarray(inp["b2"], np.float32).reshape(128, 1).copy(),
        "b3": np.full((128, 1), np.float32(np.asarray(inp["b3"])[0]), np.float32),
    }

    maps = []
    for c in range(NCORES):
        xfw, xbw = _x_windows(x, c)
        # maskT[p, g, i] = valid(head = 96c+i, dep = 128g+p)
        jj = (np.arange(128)[:, None, None] + 128 * np.arange(JB)[None, :, None])
        ii = c * OWN + np.arange(OWN)[None, None, :]
        m = ((jj >= 1) & (jj != ii)).astype(np.float32)
        d = dict(shared)
        d["xf"] = xfw
        d["xb"] = xbw
        d["frzf"] = _freeze_row(c, 0, None)
        d["frzb"] = _freeze_row(c, None, NCORES - 1)
        d["maskT"] = m
        maps.append(d)
    return maps


def run_spmd(inputs, debug=False, trace=False):
    nc = _get_nc(debug=debug)
    maps = _in_maps(inputs)
    return run_bass_kernel_spmd(nc, maps, core_ids=list(range(NCORES)),
                                trace=trace)


def kernel(**inputs):
    res = run_spmd(inputs)

    score = np.zeros((N, N), np.float32)
    colsum = np.zeros((N,), np.float32)
    for c in range(NCORES):
        st = res.results[c]["scoreT"].reshape(128, JB, OWN)
        score[c * OWN:(c + 1) * OWN] = st.transpose(2, 1, 0).reshape(OWN, N)
        cs = res.results[c]["colsumT"]          # [128, JB]
        colsum += cs.T.reshape(N)

    denom = colsum - np.float32(1.0)
    tree = np.asarray(inputs["tree"])
    v1, v2 = tree[1:, 0], tree[1:, 1]
    loss = np.float32(np.mean(np.log(denom[v2]) - score[v1, v2], dtype=np.float32))
    return loss, score


# revision 18
# speedup vs baseline: 3533.8130x; 3533.8130x over previous
# Trainium2 Bass kernel for nn_DependencyParser (2-layer biLSTM + edge-grid MLP).
#
# Strategy (8 NeuronCores):
#  - The n x n score grid is sharded row-wise: core c owns head rows [96c, 96c+96).
#  - The sequential biLSTM is time-chunked: core c computes LSTM states for its
#    own 96-step window after a 48-step warmup from zero state (the LSTM state
#    is contractive, so the warmup converges to the exact trajectory to ~1e-7).
#    Cores 0/7 have no real warmup; their fictitious warmup steps are "frozen"
#    by adding -40 to the i/o gate pre-activations so h,c stay ~0 (exact to fp32).
#  - After each layer, the owned windows are AllGathered so every core has the
#    full sequence for the next layer's input transform / the grid MLP.
#  - Recurrent matvec on the PE (bf16 weights); gates/states fp32; grid MLP in
#    float32r (full-rate fp32 matmul mode). The last matvec (w3) uses h2 as the
#    stationary operand so scores come out j-major (host un-transposes).

import numpy as np
import ml_dtypes

import concourse.bacc as bacc
import concourse.bass as bass
import concourse.mybir as mybir
import concourse.tile as tile
from concourse.bass import ds
from concourse.bass_utils import run_bass_kernel_spmd

AF = mybir.ActivationFunctionType
ALU = mybir.AluOpType
AX = mybir.AxisListType
F32 = mybir.dt.float32
F32R = mybir.dt.float32r
BF16 = mybir.dt.bfloat16

N = 768          # sequence length / grid size
EMB = 200        # input embedding dim
H = 256          # LSTM hidden per direction
NCORES = 8
OWN = N // NCORES   # 96 owned time steps / head rows per core
WARM = 32           # warmup steps for chunked recurrence
CHL = OWN + WARM    # chain length per core per direction (128)
FREEZE = -40.0      # gate preactivation offset that freezes the LSTM state
JB = N // 128       # 6 j-blocks of 128
NT = 2
JT = N // NT        # 384

# device gate layout: 8 col-blocks of 128 = [i0 i1 f0 f1 o0 o1 g0 g1]
_PERM_BLOCKS = [0, 1, 2, 3, 6, 7, 4, 5]   # source block (pytorch i,f,g,o order)
_FREEZE_M = (0, 1, 4, 5)                  # i and o col-blocks get FREEZE in warmup
PERM = np.concatenate([np.arange(b * 128, (b + 1) * 128) for b in _PERM_BLOCKS])


# ----------------------------------------------------------- host weight prep

def _whhT(whh):
    """[4H, H] -> [128, 2*8*128] bf16 PE lhsT blocks."""
    wp = np.asarray(whh, np.float32)[PERM]
    out = np.zeros((128, 2, 8, 128), np.float32)
    for kc in range(2):
        for m in range(8):
            out[:, kc, m, :] = wp[m * 128:(m + 1) * 128, kc * 128:(kc + 1) * 128].T
    return out.reshape(128, -1).astype(ml_dtypes.bfloat16)


def _wihT(wih, bih, bhh, din, nkc):
    """[4H, din] -> [128, nkc*8*128] fp32 lhsT; bias row at din, freeze at din+1."""
    wp = np.asarray(wih, np.float32)[PERM]
    bp = (np.asarray(bih, np.float32) + np.asarray(bhh, np.float32))[PERM]
    aug = np.zeros((1024, nkc * 128), np.float32)
    aug[:, :din] = wp
    aug[:, din] = bp
    for m in _FREEZE_M:
        aug[m * 128:(m + 1) * 128, din + 1] = FREEZE
    out = np.zeros((128, nkc, 8, 128), np.float32)
    for kc in range(nkc):
        for m in range(8):
            out[:, kc, m, :] = aug[m * 128:(m + 1) * 128, kc * 128:(kc + 1) * 128].T
    return out.reshape(128, -1)


def _x_windows(x, core):
    """Per-core augmented input windows, fp32 [128, 2, CHL] for f and b dirs."""
    x = np.asarray(x, np.float32)
    outs = []
    for t0 in (core * OWN - WARM, core * OWN):
        w = np.zeros((256, CHL), np.float32)
        for tt in range(CHL):
            t = t0 + tt
            if 0 <= t < N:
                w[:EMB, tt] = x[t]
                w[EMB, tt] = 1.0
            else:
                w[EMB, tt] = 1.0
                w[EMB + 1, tt] = 1.0
        outs.append(w.reshape(2, 128, CHL).transpose(1, 0, 2).copy())
    return outs  # [xf, xb]


def _w1T(w1half, b1=None):
    """[256, 512] -> [128, 5*2*128] fp32 lhsT blocks (+bias chunk rows if b1)."""
    w = np.asarray(w1half, np.float32)
    out = np.zeros((128, 5, 2, 128), np.float32)
    for kc in range(4):
        for m in range(2):
            out[:, kc, m, :] = w[m * 128:(m + 1) * 128, kc * 128:(kc + 1) * 128].T
    if b1 is not None:
        b = np.asarray(b1, np.float32)
        for m in range(2):
            out[0, 4, m, :] = b[m * 128:(m + 1) * 128]
    return out.reshape(128, -1)


def _freeze_row(core, lo_frozen_core, hi_frozen_core):
    """[1, CHL] warmup-freeze indicator rows for phase-B aug chunk."""
    r = np.zeros((1, CHL), np.float32)
    if core == lo_frozen_core:
        r[0, :WARM] = 1.0
    if core == hi_frozen_core:
        r[0, CHL - WARM:] = 1.0
    return r


# --------------------------------------------------------------- bass program

def build(debug=False, sim_nocc=False):
    """sim_nocc=True replaces collectives with equivalent local DMA traffic so
    the single-core TimelineSim cost model can run the program (timing only)."""
    nc = bacc.Bacc("TRN2", target_bir_lowering=False, debug=False,
                   num_devices=NCORES)

    def din(name, shape, dt=F32):
        return nc.dram_tensor(name, shape, dt, kind="ExternalInput")

    i_xf = din("xf", [128, 2, CHL])
    i_xb = din("xb", [128, 2, CHL])
    i_wih0f = din("wih0f", [128, 2 * 8 * 128])
    i_wih0b = din("wih0b", [128, 2 * 8 * 128])
    i_wih1f = din("wih1f", [128, 5 * 8 * 128])
    i_wih1b = din("wih1b", [128, 5 * 8 * 128])
    i_whh = {d: din(f"whh{d}", [128, 2 * 8 * 128], BF16)
             for d in ("0f", "0b", "1f", "1b")}
    i_frzf = din("frzf", [1, CHL])
    i_frzb = din("frzb", [1, CHL])
    i_w1h = din("w1h", [128, 5 * 2 * 128], F32R)
    i_w1c = din("w1c", [128, 4 * 2 * 128], F32R)
    i_w2 = din("w2", [128, 2 * 128], F32R)
    i_w3 = din("w3", [128, 1], BF16)
    i_b2 = din("b2", [128, 1])
    i_b3 = din("b3", [128, 1])
    i_mask = din("maskT", [128, JB, OWN])

    o_score = nc.dram_tensor("scoreT", [128, JB * OWN], F32, kind="ExternalOutput")
    o_colsum = nc.dram_tensor("colsumT", [128, JB], F32, kind="ExternalOutput")
    dbg = {}
    if debug:
        for nm, shp, dt in (("d_xw0f", [128, 8 * CHL], F32),
                            ("d_h0", [128, 4 * (N + 2 * WARM)], F32),
                            ("d_xw1f", [128, 8 * CHL], F32),
                            ("d_emb", [128, 4 * N], F32R),
                            ("d_aT", [128, 2 * OWN], F32),
                            ("d_cT", [128, 2 * N], F32)):
            dbg[nm] = nc.dram_tensor(nm, shp, dt, kind="ExternalOutput")

    HPAD = N + 2 * WARM

    with tile.TileContext(nc) as tc:
        with (
            tc.tile_pool(name="persist", bufs=1) as P,
            tc.tile_pool(name="mmp", bufs=2, space="PSUM") as MMP,
            tc.tile_pool(name="dram", bufs=1, space="DRAM") as DR,
        ):
            # ---- load static inputs to SBUF
            xf = P.tile([128, 2, CHL], F32)
            nc.sync.dma_start(xf[:], i_xf[:])
            xb = P.tile([128, 2, CHL], F32)
            nc.sync.dma_start(xb[:], i_xb[:])
            wih0 = {}
            for d, inp in (("f", i_wih0f), ("b", i_wih0b)):
                t = P.tile([128, 2, 8, 128], F32, tag=f"wih0{d}")
                nc.sync.dma_start(t[:], inp[:].rearrange("p (a b c) -> p a b c", a=2, b=8))
                wih0[d] = t
            wih1 = {}
            for d, inp in (("f", i_wih1f), ("b", i_wih1b)):
                t = P.tile([128, 5, 8, 128], F32, tag=f"wih1{d}")
                nc.sync.dma_start(t[:], inp[:].rearrange("p (a b c) -> p a b c", a=5, b=8))
                wih1[d] = t
            whh = {}
            for d in ("0f", "0b", "1f", "1b"):
                t = P.tile([128, 2, 8, 128], BF16, tag=f"whh{d}")
                nc.sync.dma_start(t[:], i_whh[d][:].rearrange("p (a b c) -> p a b c", a=2, b=8))
                whh[d] = t
            w1h = P.tile([128, 5, 2, 128], F32R)
            nc.sync.dma_start(w1h[:], i_w1h[:].rearrange("p (a b c) -> p a b c", a=5, b=2))
            w1c = P.tile([128, 4, 2, 128], F32R)
            nc.sync.dma_start(w1c[:], i_w1c[:].rearrange("p (a b c) -> p a b c", a=4, b=2))
            w2 = P.tile([128, 2, 128], F32R)
            nc.sync.dma_start(w2[:], i_w2[:].rearrange("p (a b) -> p a b", a=2))
            w3 = P.tile([128, 1], BF16)
            nc.sync.dma_start(w3[:], i_w3[:])
            b2 = P.tile([128, 1], F32)
            nc.sync.dma_start(b2[:], i_b2[:])
            b3 = P.tile([128, 1], F32)
            nc.sync.dma_start(b3[:], i_b3[:])
            maskT = P.tile([128, JB, OWN], F32)
            nc.sync.dma_start(maskT[:], i_mask[:])

            pid = nc.vector.partition_id()

            def xw_from(lhsT, rhs, nkc, xw_out):
                """xw_out[128, 8, CHL] fp32 = sum_kc lhsT[:,kc,m,:].T @ rhs[:,kc,:]"""
                for m in range(8):
                    ps = MMP.tile([128, CHL], F32, tag="mmp")
                    for kc in range(nkc):
                        nc.tensor.matmul(ps[:], lhsT[:, kc, m, :], rhs[:, kc, :],
                                         start=(kc == 0), stop=(kc == nkc - 1))
                    nc.vector.tensor_copy(xw_out[:, m, :], ps[:])

            xw_f = P.tile([128, 8, CHL], F32)
            xw_b = P.tile([128, 8, CHL], F32)
            h_ownT = P.tile([128, 4, OWN], F32)
            emb_ownT = P.tile([128, 4, OWN], F32R)
            h0_pad = P.tile([128, 4, HPAD], F32)

            with (
                tc.tile_pool(name="small", bufs=3) as SM,
                tc.tile_pool(name="hp", bufs=2) as HP,
                tc.tile_pool(name="gpsf", bufs=2, space="PSUM") as GPSF,
                tc.tile_pool(name="gpsb", bufs=2, space="PSUM") as GPSB,
            ):
                def chain(name, whhT, xw_sb, gpool, gtag, storeT, store_base, rev):
                    """One LSTM chain of CHL steps; owned h written to storeT."""
                    TC = P.tile([128, 4], F32, tag=f"TC_{name}")  # [tg0 tg1 c0 c1]
                    nc.vector.memset(TC[:, 2:4], 0.0)
                    h_bf = HP.tile([128, 2], BF16, tag=f"h_{name}")
                    nc.vector.memset(h_bf[:], 0.0)
                    for ss in range(CHL):
                        col = (CHL - 1 - ss) if rev else ss
                        gps = gpool.tile([128, 8], F32, tag=gtag)
                        for m in range(8):
                            nc.tensor.matmul(gps[:, m:m + 1], whhT[:, 0, m, :],
                                             h_bf[:, 0:1], start=True, stop=False)
                            nc.tensor.matmul(gps[:, m:m + 1], whhT[:, 1, m, :],
                                             h_bf[:, 1:2], start=False, stop=True)
                        gates = SM.tile([128, 8], F32, tag=f"g_{name}")
                        nc.vector.tensor_tensor(gates[:], gps[:], xw_sb[:, :, col],
                                                ALU.add)
                        S = SM.tile([128, 6], F32, tag=f"S_{name}")
                        nc.scalar.activation(S[:], gates[:, 0:6], AF.Sigmoid)
                        nc.scalar.activation(TC[:, 0:2], gates[:, 6:8], AF.Tanh)
                        Pp = SM.tile([128, 4], F32, tag=f"P_{name}")
                        nc.vector.tensor_tensor(Pp[:], S[:, 0:4], TC[:], ALU.mult)
                        nc.vector.tensor_tensor(TC[:, 2:4], Pp[:, 0:2], Pp[:, 2:4],
                                                ALU.add)
                        T2 = SM.tile([128, 2], F32, tag=f"T2_{name}")
                        nc.scalar.activation(T2[:], TC[:, 2:4], AF.Tanh)
                        h_bf = HP.tile([128, 2], BF16, tag=f"h_{name}")
                        nc.vector.tensor_tensor(h_bf[:], S[:, 4:6], T2[:], ALU.mult)
                        if ss >= WARM:
                            idx = (ss - WARM) if not rev else (CHL - 1 - ss)
                            nc.vector.tensor_tensor(
                                storeT[:, store_base:store_base + 2, idx:idx + 1],
                                S[:, 4:6], T2[:], ALU.mult)

                # ========== phase A: layer 0 ==========
                xw_from(wih0["f"], xf, 2, xw_f)
                xw_from(wih0["b"], xb, 2, xw_b)
                if debug:
                    nc.sync.dma_start(dbg["d_xw0f"][:],
                                      xw_f[:].rearrange("p a b -> p (a b)"))

                chain("a_f", whh["0f"], xw_f, GPSF, "gf", h_ownT, 0, rev=False)
                chain("a_b", whh["0b"], xw_b, GPSB, "gb", h_ownT, 2, rev=True)

                cc1_in = DR.tile([4, 128, OWN], F32)
                cc1_out = DR.tile([NCORES, 4, 128, OWN], F32,
                                  addr_space="Local" if sim_nocc else "Shared")
                nc.sync.dma_start(cc1_in[:].rearrange("c p k -> p c k"), h_ownT[:])
                if sim_nocc:
                    for b in range(NCORES):
                        nc.sync.dma_start(cc1_out[b], cc1_in[:])
                else:
                    nc.gpsimd.collective_compute(
                        "AllGather", ALU.bypass,
                        replica_groups=[list(range(NCORES))],
                        ins=[cc1_in.opt()], outs=[cc1_out.opt()])

                nc.vector.memset(h0_pad[:, :, 0:WARM], 0.0)
                nc.vector.memset(h0_pad[:, :, WARM + N:], 0.0)
                for ch in range(4):
                    nc.sync.dma_start(
                        h0_pad[:, ch, WARM:WARM + N].rearrange("p (b k) -> p b k", k=OWN),
                        cc1_out[:, ch, :, :].rearrange("b p k -> p b k"))
                if debug:
                    nc.sync.dma_start(dbg["d_h0"][:],
                                      h0_pad[:].rearrange("p a b -> p (a b)"))

                # ========== phase B: layer 1 ==========
                h0w = {}
                for d, off, frz in (("f", pid * OWN, i_frzf),
                                    ("b", pid * OWN + WARM, i_frzb)):
                    hw = P.tile([128, 5, CHL], F32, tag=f"h0w_{d}")
                    nc.vector.tensor_copy(hw[:, 0:4, :], h0_pad[:, :, ds(off, CHL)])
                    nc.vector.memset(hw[:, 4, :], 0.0)
                    nc.vector.memset(hw[0:1, 4, :], 1.0)
                    nc.sync.dma_start(hw[1:2, 4, :], frz[:])
                    h0w[d] = hw

                xw_from(wih1["f"], h0w["f"], 5, xw_f)
                xw_from(wih1["b"], h0w["b"], 5, xw_b)
                if debug:
                    nc.sync.dma_start(dbg["d_xw1f"][:],
                                      xw_f[:].rearrange("p a b -> p (a b)"))

                chain("b_f", whh["1f"], xw_f, GPSF, "gf", emb_ownT, 0, rev=False)
                chain("b_b", whh["1b"], xw_b, GPSB, "gb", emb_ownT, 2, rev=True)

            # ---- AllGather embeddings
            cc2_in = DR.tile([4, 128, OWN], F32R)
            cc2_out = DR.tile([NCORES, 4, 128, OWN], F32R,
                              addr_space="Local" if sim_nocc else "Shared")
            nc.sync.dma_start(cc2_in[:].rearrange("c p k -> p c k"), emb_ownT[:])
            if sim_nocc:
                for b in range(NCORES):
                    nc.sync.dma_start(cc2_out[b], cc2_in[:])
            else:
                nc.gpsimd.collective_compute(
                    "AllGather", ALU.bypass,
                    replica_groups=[list(range(NCORES))],
                    ins=[cc2_in.opt()], outs=[cc2_out.opt()])

            embT = P.tile([128, 4, N], F32R)
            for ch in range(4):
                nc.sync.dma_start(
                    embT[:, ch, :].rearrange("p (b k) -> p b k", k=OWN),
                    cc2_out[:, ch, :, :].rearrange("b p k -> p b k"))
            if debug:
                nc.sync.dma_start(dbg["d_emb"][:],
                                  embT[:].rearrange("p a b -> p (a b)"))

            # ========== grid MLP (scores come out j-major) ==========
            with (
                tc.tile_pool(name="gridsm", bufs=3) as GSM,
                tc.tile_pool(name="rawp", bufs=1, space="PSUM") as RAWP,
            ):
                ones1f = P.tile([1, OWN], F32)
                nc.vector.memset(ones1f[:], 1.0)
                ones1 = P.tile([1, OWN], F32R)
                nc.vector.tensor_copy(ones1[:], ones1f[:])

                aT = P.tile([128, 2, OWN], F32)
                for m in range(2):
                    ps = MMP.tile([128, OWN], F32, tag="mmp")
                    for kc in range(4):
                        nc.tensor.matmul(ps[:], w1h[:, kc, m, :], emb_ownT[:, kc, :],
                                         start=(kc == 0), stop=False)
                    nc.tensor.matmul(ps[:], w1h[0:1, 4, m, :], ones1[:],
                                     start=False, stop=True)
                    nc.vector.tensor_copy(aT[:, m, :], ps[:])
                if debug:
                    nc.sync.dma_start(dbg["d_aT"][:],
                                      aT[:].rearrange("p a b -> p (a b)"))

                cT = P.tile([128, 2, N], F32)
                for m in range(2):
                    for nt in range(NT):
                        ps = MMP.tile([128, JT], F32, tag="mmp")
                        for kc in range(4):
                            nc.tensor.matmul(ps[:], w1c[:, kc, m, :],
                                             embT[:, kc, nt * JT:(nt + 1) * JT],
                                             start=(kc == 0), stop=(kc == 3))
                        nc.vector.tensor_copy(cT[:, m, nt * JT:(nt + 1) * JT], ps[:])
                if debug:
                    nc.sync.dma_start(dbg["d_cT"][:],
                                      cT[:].rearrange("p a b -> p (a b)"))

                rawT = RAWP.tile([128, JB, OWN], F32)
                for i in range(OWN):
                    for nt in range(NT):
                        h1 = GSM.tile([128, 2, JT], F32R, tag="h1")
                        for m in range(2):
                            nc.vector.tensor_scalar(
                                h1[:, m, :], cT[:, m, nt * JT:(nt + 1) * JT],
                                aT[:, m, i:i + 1], 0.0, op0=ALU.add, op1=ALU.max)
                        h2ps = MMP.tile([128, JT], F32, tag="mmp")
                        nc.tensor.matmul(h2ps[:], w2[:, 0, :], h1[:, 0, :],
                                         start=True, stop=False)
                        nc.tensor.matmul(h2ps[:], w2[:, 1, :], h1[:, 1, :],
                                         start=False, stop=True)
                        h2r = GSM.tile([128, JT], BF16, tag="h2r")
                        nc.scalar.activation(h2r[:], h2ps[:], AF.Relu, bias=b2[:, 0:1])
                        for jb in range(3):
                            g = nt * 3 + jb
                            nc.tensor.matmul(rawT[:, g, i:i + 1],
                                             h2r[:, jb * 128:(jb + 1) * 128], w3[:],
                                             start=True, stop=True)

                scoreT = P.tile([128, JB, OWN], F32)
                nc.scalar.activation(scoreT[:].rearrange("p a b -> p (a b)"),
                                     rawT[:].rearrange("p a b -> p (a b)"),
                                     AF.Relu, bias=b3[:, 0:1])
                scoreM = P.tile([128, JB, OWN], F32)
                nc.vector.tensor_tensor(scoreM[:], scoreT[:], maskT[:], ALU.mult)
                nc.sync.dma_start(o_score[:],
                                  scoreM[:].rearrange("p a b -> p (a b)"))

                esT = P.tile([128, JB, OWN], F32)
                nc.scalar.activation(esT[:].rearrange("p a b -> p (a b)"),
                                     scoreM[:].rearrange("p a b -> p (a b)"), AF.Exp)
                csT = P.tile([128, JB], F32)
                for g in range(JB):
                    nc.vector.tensor_reduce(csT[:, g:g + 1], esT[:, g, :],
                                            axis=AX.X, op=ALU.add)
                nc.sync.dma_start(o_colsum[:], csT[:])

    nc.compile()
    return nc


# ------------------------------------------------------------------ host glue

_NC_CACHE = {}


def _get_nc(debug=False):
    key = bool(debug)
    if key not in _NC_CACHE:
        _NC_CACHE[key] = build(debug)
    return _NC_CACHE[key]


def _in_maps(inputs):
    inp = {k: np.asarray(v) for k, v in inputs.items()}
    x = inp["x"].astype(np.float32)

    shared = {
        "wih0f": _wihT(inp["wih_l0f"], inp["bih_l0f"], inp["bhh_l0f"], EMB, 2),
        "wih0b": _wihT(inp["wih_l0b"], inp["bih_l0b"], inp["bhh_l0b"], EMB, 2),
        "wih1f": _wihT(inp["wih_l1f"], inp["bih_l1f"], inp["bhh_l1f"], 2 * H, 5),
        "wih1b": _wihT(inp["wih_l1b"], inp["bih_l1b"], inp["bhh_l1b"], 2 * H, 5),
        "whh0f": _whhT(inp["whh_l0f"]), "whh0b": _whhT(inp["whh_l0b"]),
        "whh1f": _whhT(inp["whh_l1f"]), "whh1b": _whhT(inp["whh_l1b"]),
        "w1h": _w1T(inp["w1"][:, :2 * H], inp["b1"]),
        "w1c": _w1T(inp["w1"][:, 2 * H:])[:, :4 * 2 * 128],
        "w2": np.ascontiguousarray(
            np.asarray(inp["w2"], np.float32).T.reshape(2, 128, 128)
            .transpose(1, 0, 2)).reshape(128, 256),
        "w3": np.asarray(inp["w3"], np.float32).reshape(128, 1).astype(
            ml_dtypes.bfloat16),
        "b2": np.asarray(inp["b2"], np.float32).reshape(128, 1).copy(),
        "b3": np.full((128, 1), np.float32(np.asarray(inp["b3"])[0]), np.float32),
    }

    maps = []
    for c in range(NCORES):
        xfw, xbw = _x_windows(x, c)
        # maskT[p, g, i] = valid(head = 96c+i, dep = 128g+p)
        jj = (np.arange(128)[:, None, None] + 128 * np.arange(JB)[None, :, None])
        ii = c * OWN + np.arange(OWN)[None, None, :]
        m = ((jj >= 1) & (jj != ii)).astype(np.float32)
        d = dict(shared)
        d["xf"] = xfw
        d["xb"] = xbw
        d["frzf"] = _freeze_row(c, 0, None)
        d["frzb"] = _freeze_row(c, None, NCORES - 1)
        d["maskT"] = m
        maps.append(d)
    return maps


def run_spmd(inputs, debug=False, trace=False):
    nc = _get_nc(debug=debug)
    maps = _in_maps(inputs)
    return run_bass_kernel_spmd(nc, maps, core_ids=list(range(NCORES)),
                                trace=trace)


def kernel(**inputs):
    res = run_spmd(inputs)

    score = np.zeros((N, N), np.float32)
    colsum = np.zeros((N,), np.float32)
    for c in range(NCORES):
        st = res.results[c]["scoreT"].reshape(128, JB, OWN)
        score[c * OWN:(c + 1) * OWN] = st.transpose(2, 1, 0).reshape(OWN, N)
        cs = res.results[c]["colsumT"]          # [128, JB]
        colsum += cs.T.reshape(N)

    denom = colsum - np.float32(1.0)
    tree = np.asarray(inputs["tree"])
    v1, v2 = tree[1:, 0], tree[1:, 1]
    loss = np.float32(np.mean(np.log(denom[v2]) - score[v1, v2], dtype=np.float32))
    return loss, score


# revision 19
# speedup vs baseline: 3705.7670x; 1.0487x over previous
# Trainium2 Bass kernel for nn_DependencyParser (2-layer biLSTM + edge-grid MLP).
#
# Strategy (8 NeuronCores):
#  - The n x n score grid is sharded row-wise: core c owns head rows [96c, 96c+96).
#  - The sequential biLSTM is time-chunked: core c computes LSTM states for its
#    own 96-step window after a 48-step warmup from zero state (the LSTM state
#    is contractive, so the warmup converges to the exact trajectory to ~1e-7).
#    Cores 0/7 have no real warmup; their fictitious warmup steps are "frozen"
#    by adding -40 to the i/o gate pre-activations so h,c stay ~0 (exact to fp32).
#  - After each layer, the owned windows are AllGathered so every core has the
#    full sequence for the next layer's input transform / the grid MLP.
#  - Recurrent matvec on the PE (bf16 weights); gates/states fp32; grid MLP in
#    float32r (full-rate fp32 matmul mode). The last matvec (w3) uses h2 as the
#    stationary operand so scores come out j-major (host un-transposes).

import numpy as np
import ml_dtypes

import concourse.bacc as bacc
import concourse.bass as bass
import concourse.mybir as mybir
import concourse.tile as tile
from concourse.bass import ds
from concourse.bass_utils import run_bass_kernel_spmd

AF = mybir.ActivationFunctionType
ALU = mybir.AluOpType
AX = mybir.AxisListType
F32 = mybir.dt.float32
F32R = mybir.dt.float32r
BF16 = mybir.dt.bfloat16

N = 768          # sequence length / grid size
EMB = 200        # input embedding dim
H = 256          # LSTM hidden per direction
NCORES = 8
OWN = N // NCORES   # 96 owned time steps / head rows per core
WARM = 24           # warmup steps for chunked recurrence
CHL = OWN + WARM    # chain length per core per direction (120)
FREEZE = -40.0      # gate preactivation offset that freezes the LSTM state
JB = N // 128       # 6 j-blocks of 128
NT = 2
JT = N // NT        # 384

# device gate layout: 8 col-blocks of 128 = [i0 i1 f0 f1 o0 o1 g0 g1]
_PERM_BLOCKS = [0, 1, 2, 3, 6, 7, 4, 5]   # source block (pytorch i,f,g,o order)
_FREEZE_M = (0, 1, 4, 5)                  # i and o col-blocks get FREEZE in warmup
PERM = np.concatenate([np.arange(b * 128, (b + 1) * 128) for b in _PERM_BLOCKS])


# ----------------------------------------------------------- host weight prep

def _whhT(whh):
    """[4H, H] -> [128, 2*8*128] bf16 PE lhsT blocks."""
    wp = np.asarray(whh, np.float32)[PERM]
    out = np.zeros((128, 2, 8, 128), np.float32)
    for kc in range(2):
        for m in range(8):
            out[:, kc, m, :] = wp[m * 128:(m + 1) * 128, kc * 128:(kc + 1) * 128].T
    return out.reshape(128, -1).astype(ml_dtypes.bfloat16)


def _wihT(wih, bih, bhh, din, nkc):
    """[4H, din] -> [128, nkc*8*128] fp32 lhsT; bias row at din, freeze at din+1."""
    wp = np.asarray(wih, np.float32)[PERM]
    bp = (np.asarray(bih, np.float32) + np.asarray(bhh, np.float32))[PERM]
    aug = np.zeros((1024, nkc * 128), np.float32)
    aug[:, :din] = wp
    aug[:, din] = bp
    for m in _FREEZE_M:
        aug[m * 128:(m + 1) * 128, din + 1] = FREEZE
    out = np.zeros((128, nkc, 8, 128), np.float32)
    for kc in range(nkc):
        for m in range(8):
            out[:, kc, m, :] = aug[m * 128:(m + 1) * 128, kc * 128:(kc + 1) * 128].T
    return out.reshape(128, -1)


def _x_windows(x, core):
    """Per-core augmented input windows, fp32 [128, 2, CHL] for f and b dirs."""
    x = np.asarray(x, np.float32)
    outs = []
    for t0 in (core * OWN - WARM, core * OWN):
        w = np.zeros((256, CHL), np.float32)
        for tt in range(CHL):
            t = t0 + tt
            if 0 <= t < N:
                w[:EMB, tt] = x[t]
                w[EMB, tt] = 1.0
            else:
                w[EMB, tt] = 1.0
                w[EMB + 1, tt] = 1.0
        outs.append(w.reshape(2, 128, CHL).transpose(1, 0, 2).copy())
    return outs  # [xf, xb]


def _w1T(w1half, b1=None):
    """[256, 512] -> [128, 5*2*128] fp32 lhsT blocks (+bias chunk rows if b1)."""
    w = np.asarray(w1half, np.float32)
    out = np.zeros((128, 5, 2, 128), np.float32)
    for kc in range(4):
        for m in range(2):
            out[:, kc, m, :] = w[m * 128:(m + 1) * 128, kc * 128:(kc + 1) * 128].T
    if b1 is not None:
        b = np.asarray(b1, np.float32)
        for m in range(2):
            out[0, 4, m, :] = b[m * 128:(m + 1) * 128]
    return out.reshape(128, -1)


def _freeze_row(core, lo_frozen_core, hi_frozen_core):
    """[1, CHL] warmup-freeze indicator rows for phase-B aug chunk."""
    r = np.zeros((1, CHL), np.float32)
    if core == lo_frozen_core:
        r[0, :WARM] = 1.0
    if core == hi_frozen_core:
        r[0, CHL - WARM:] = 1.0
    return r


# --------------------------------------------------------------- bass program

def build(debug=False, sim_nocc=False):
    """sim_nocc=True replaces collectives with equivalent local DMA traffic so
    the single-core TimelineSim cost model can run the program (timing only)."""
    nc = bacc.Bacc("TRN2", target_bir_lowering=False, debug=False,
                   num_devices=NCORES)

    def din(name, shape, dt=F32):
        return nc.dram_tensor(name, shape, dt, kind="ExternalInput")

    i_xf = din("xf", [128, 2, CHL])
    i_xb = din("xb", [128, 2, CHL])
    i_wih0f = din("wih0f", [128, 2 * 8 * 128])
    i_wih0b = din("wih0b", [128, 2 * 8 * 128])
    i_wih1f = din("wih1f", [128, 5 * 8 * 128])
    i_wih1b = din("wih1b", [128, 5 * 8 * 128])
    i_whh = {d: din(f"whh{d}", [128, 2 * 8 * 128], BF16)
             for d in ("0f", "0b", "1f", "1b")}
    i_frzf = din("frzf", [1, CHL])
    i_frzb = din("frzb", [1, CHL])
    i_w1h = din("w1h", [128, 5 * 2 * 128], F32R)
    i_w1c = din("w1c", [128, 4 * 2 * 128], F32R)
    i_w2 = din("w2", [128, 2 * 128], F32R)
    i_w3 = din("w3", [128, 1], BF16)
    i_b2 = din("b2", [128, 1])
    i_b3 = din("b3", [128, 1])
    i_mask = din("maskT", [128, JB, OWN])

    o_score = nc.dram_tensor("scoreT", [128, JB * OWN], F32, kind="ExternalOutput")
    o_colsum = nc.dram_tensor("colsumT", [128, JB], F32, kind="ExternalOutput")
    dbg = {}
    if debug:
        for nm, shp, dt in (("d_xw0f", [128, 8 * CHL], F32),
                            ("d_h0", [128, 4 * (N + 2 * WARM)], F32),
                            ("d_xw1f", [128, 8 * CHL], F32),
                            ("d_emb", [128, 4 * N], F32R),
                            ("d_aT", [128, 2 * OWN], F32),
                            ("d_cT", [128, 2 * N], F32)):
            dbg[nm] = nc.dram_tensor(nm, shp, dt, kind="ExternalOutput")

    HPAD = N + 2 * WARM

    with tile.TileContext(nc) as tc:
        with (
            tc.tile_pool(name="persist", bufs=1) as P,
            tc.tile_pool(name="mmp", bufs=2, space="PSUM") as MMP,
            tc.tile_pool(name="dram", bufs=1, space="DRAM") as DR,
        ):
            # ---- load static inputs to SBUF
            xf = P.tile([128, 2, CHL], F32)
            nc.sync.dma_start(xf[:], i_xf[:])
            xb = P.tile([128, 2, CHL], F32)
            nc.sync.dma_start(xb[:], i_xb[:])
            wih0 = {}
            for d, inp in (("f", i_wih0f), ("b", i_wih0b)):
                t = P.tile([128, 2, 8, 128], F32, tag=f"wih0{d}")
                nc.sync.dma_start(t[:], inp[:].rearrange("p (a b c) -> p a b c", a=2, b=8))
                wih0[d] = t
            wih1 = {}
            for d, inp in (("f", i_wih1f), ("b", i_wih1b)):
                t = P.tile([128, 5, 8, 128], F32, tag=f"wih1{d}")
                nc.sync.dma_start(t[:], inp[:].rearrange("p (a b c) -> p a b c", a=5, b=8))
                wih1[d] = t
            whh = {}
            for d in ("0f", "0b", "1f", "1b"):
                t = P.tile([128, 2, 8, 128], BF16, tag=f"whh{d}")
                nc.sync.dma_start(t[:], i_whh[d][:].rearrange("p (a b c) -> p a b c", a=2, b=8))
                whh[d] = t
            w1h = P.tile([128, 5, 2, 128], F32R)
            nc.sync.dma_start(w1h[:], i_w1h[:].rearrange("p (a b c) -> p a b c", a=5, b=2))
            w1c = P.tile([128, 4, 2, 128], F32R)
            nc.sync.dma_start(w1c[:], i_w1c[:].rearrange("p (a b c) -> p a b c", a=4, b=2))
            w2 = P.tile([128, 2, 128], F32R)
            nc.sync.dma_start(w2[:], i_w2[:].rearrange("p (a b) -> p a b", a=2))
            w3 = P.tile([128, 1], BF16)
            nc.sync.dma_start(w3[:], i_w3[:])
            b2 = P.tile([128, 1], F32)
            nc.sync.dma_start(b2[:], i_b2[:])
            b3 = P.tile([128, 1], F32)
            nc.sync.dma_start(b3[:], i_b3[:])
            maskT = P.tile([128, JB, OWN], F32)
            nc.sync.dma_start(maskT[:], i_mask[:])

            pid = nc.vector.partition_id()

            def xw_from(lhsT, rhs, nkc, xw_out):
                """xw_out[128, 8, CHL] fp32 = sum_kc lhsT[:,kc,m,:].T @ rhs[:,kc,:]"""
                for m in range(8):
                    ps = MMP.tile([128, CHL], F32, tag="mmp")
                    for kc in range(nkc):
                        nc.tensor.matmul(ps[:], lhsT[:, kc, m, :], rhs[:, kc, :],
                                         start=(kc == 0), stop=(kc == nkc - 1))
                    nc.vector.tensor_copy(xw_out[:, m, :], ps[:])

            xw_f = P.tile([128, 8, CHL], F32)
            xw_b = P.tile([128, 8, CHL], F32)
            h_ownT = P.tile([128, 4, OWN], F32)
            emb_ownT = P.tile([128, 4, OWN], F32R)
            h0_pad = P.tile([128, 4, HPAD], F32)

            with (
                tc.tile_pool(name="small", bufs=3) as SM,
                tc.tile_pool(name="hp", bufs=2) as HP,
                tc.tile_pool(name="gpsf", bufs=2, space="PSUM") as GPSF,
                tc.tile_pool(name="gpsb", bufs=2, space="PSUM") as GPSB,
            ):
                def chain(name, whhT, xw_sb, gpool, gtag, storeT, store_base, rev):
                    """One LSTM chain of CHL steps; owned h written to storeT."""
                    TC = P.tile([128, 4], F32, tag=f"TC_{name}")  # [tg0 tg1 c0 c1]
                    nc.vector.memset(TC[:, 2:4], 0.0)
                    h_bf = HP.tile([128, 2], BF16, tag=f"h_{name}")
                    nc.vector.memset(h_bf[:], 0.0)
                    for ss in range(CHL):
                        col = (CHL - 1 - ss) if rev else ss
                        gps = gpool.tile([128, 8], F32, tag=gtag)
                        for m in range(8):
                            nc.tensor.matmul(gps[:, m:m + 1], whhT[:, 0, m, :],
                                             h_bf[:, 0:1], start=True, stop=False)
                            nc.tensor.matmul(gps[:, m:m + 1], whhT[:, 1, m, :],
                                             h_bf[:, 1:2], start=False, stop=True)
                        gates = SM.tile([128, 8], F32, tag=f"g_{name}")
                        nc.vector.tensor_tensor(gates[:], gps[:], xw_sb[:, :, col],
                                                ALU.add)
                        S = SM.tile([128, 6], F32, tag=f"S_{name}")
                        nc.scalar.activation(S[:], gates[:, 0:6], AF.Sigmoid)
                        nc.scalar.activation(TC[:, 0:2], gates[:, 6:8], AF.Tanh)
                        Pp = SM.tile([128, 4], F32, tag=f"P_{name}")
                        nc.vector.tensor_tensor(Pp[:], S[:, 0:4], TC[:], ALU.mult)
                        nc.vector.tensor_tensor(TC[:, 2:4], Pp[:, 0:2], Pp[:, 2:4],
                                                ALU.add)
                        T2 = SM.tile([128, 2], F32, tag=f"T2_{name}")
                        nc.scalar.activation(T2[:], TC[:, 2:4], AF.Tanh)
                        h_bf = HP.tile([128, 2], BF16, tag=f"h_{name}")
                        nc.vector.tensor_tensor(h_bf[:], S[:, 4:6], T2[:], ALU.mult)
                        if ss >= WARM:
                            idx = (ss - WARM) if not rev else (CHL - 1 - ss)
                            nc.vector.tensor_tensor(
                                storeT[:, store_base:store_base + 2, idx:idx + 1],
                                S[:, 4:6], T2[:], ALU.mult)

                # ========== phase A: layer 0 ==========
                xw_from(wih0["f"], xf, 2, xw_f)
                xw_from(wih0["b"], xb, 2, xw_b)
                if debug:
                    nc.sync.dma_start(dbg["d_xw0f"][:],
                                      xw_f[:].rearrange("p a b -> p (a b)"))

                chain("a_f", whh["0f"], xw_f, GPSF, "gf", h_ownT, 0, rev=False)
                chain("a_b", whh["0b"], xw_b, GPSB, "gb", h_ownT, 2, rev=True)

                cc1_in = DR.tile([4, 128, OWN], F32)
                cc1_out = DR.tile([NCORES, 4, 128, OWN], F32,
                                  addr_space="Local" if sim_nocc else "Shared")
                nc.sync.dma_start(cc1_in[:].rearrange("c p k -> p c k"), h_ownT[:])
                if sim_nocc:
                    for b in range(NCORES):
                        nc.sync.dma_start(cc1_out[b], cc1_in[:])
                else:
                    nc.gpsimd.collective_compute(
                        "AllGather", ALU.bypass,
                        replica_groups=[list(range(NCORES))],
                        ins=[cc1_in.opt()], outs=[cc1_out.opt()])

                nc.vector.memset(h0_pad[:, :, 0:WARM], 0.0)
                nc.vector.memset(h0_pad[:, :, WARM + N:], 0.0)
                for ch in range(4):
                    nc.sync.dma_start(
                        h0_pad[:, ch, WARM:WARM + N].rearrange("p (b k) -> p b k", k=OWN),
                        cc1_out[:, ch, :, :].rearrange("b p k -> p b k"))
                if debug:
                    nc.sync.dma_start(dbg["d_h0"][:],
                                      h0_pad[:].rearrange("p a b -> p (a b)"))

                # ========== phase B: layer 1 ==========
                h0w = {}
                for d, off, frz in (("f", pid * OWN, i_frzf),
                                    ("b", pid * OWN + WARM, i_frzb)):
                    hw = P.tile([128, 5, CHL], F32, tag=f"h0w_{d}")
                    nc.vector.tensor_copy(hw[:, 0:4, :], h0_pad[:, :, ds(off, CHL)])
                    nc.vector.memset(hw[:, 4, :], 0.0)
                    nc.vector.memset(hw[0:1, 4, :], 1.0)
                    nc.sync.dma_start(hw[1:2, 4, :], frz[:])
                    h0w[d] = hw

                xw_from(wih1["f"], h0w["f"], 5, xw_f)
                xw_from(wih1["b"], h0w["b"], 5, xw_b)
                if debug:
                    nc.sync.dma_start(dbg["d_xw1f"][:],
                                      xw_f[:].rearrange("p a b -> p (a b)"))

                chain("b_f", whh["1f"], xw_f, GPSF, "gf", emb_ownT, 0, rev=False)
                chain("b_b", whh["1b"], xw_b, GPSB, "gb", emb_ownT, 2, rev=True)

            # ---- AllGather embeddings
            cc2_in = DR.tile([4, 128, OWN], F32R)
            cc2_out = DR.tile([NCORES, 4, 128, OWN], F32R,
                              addr_space="Local" if sim_nocc else "Shared")
            nc.sync.dma_start(cc2_in[:].rearrange("c p k -> p c k"), emb_ownT[:])
            if sim_nocc:
                for b in range(NCORES):
                    nc.sync.dma_start(cc2_out[b], cc2_in[:])
            else:
                nc.gpsimd.collective_compute(
                    "AllGather", ALU.bypass,
                    replica_groups=[list(range(NCORES))],
                    ins=[cc2_in.opt()], outs=[cc2_out.opt()])

            embT = P.tile([128, 4, N], F32R)
            for ch in range(4):
                nc.sync.dma_start(
                    embT[:, ch, :].rearrange("p (b k) -> p b k", k=OWN),
                    cc2_out[:, ch, :, :].rearrange("b p k -> p b k"))
            if debug:
                nc.sync.dma_start(dbg["d_emb"][:],
                                  embT[:].rearrange("p a b -> p (a b)"))

            # ========== grid MLP (scores come out j-major) ==========
            with (
                tc.tile_pool(name="gridsm", bufs=3) as GSM,
                tc.tile_pool(name="rawp", bufs=1, space="PSUM") as RAWP,
            ):
                ones1f = P.tile([1, OWN], F32)
                nc.vector.memset(ones1f[:], 1.0)
                ones1 = P.tile([1, OWN], F32R)
                nc.vector.tensor_copy(ones1[:], ones1f[:])

                aT = P.tile([128, 2, OWN], F32)
                for m in range(2):
                    ps = MMP.tile([128, OWN], F32, tag="mmp")
                    for kc in range(4):
                        nc.tensor.matmul(ps[:], w1h[:, kc, m, :], emb_ownT[:, kc, :],
                                         start=(kc == 0), stop=False)
                    nc.tensor.matmul(ps[:], w1h[0:1, 4, m, :], ones1[:],
                                     start=False, stop=True)
                    nc.vector.tensor_copy(aT[:, m, :], ps[:])
                if debug:
                    nc.sync.dma_start(dbg["d_aT"][:],
                                      aT[:].rearrange("p a b -> p (a b)"))

                cT = P.tile([128, 2, N], F32)
                for m in range(2):
                    for nt in range(NT):
                        ps = MMP.tile([128, JT], F32, tag="mmp")
                        for kc in range(4):
                            nc.tensor.matmul(ps[:], w1c[:, kc, m, :],
                                             embT[:, kc, nt * JT:(nt + 1) * JT],
                                             start=(kc == 0), stop=(kc == 3))
                        nc.vector.tensor_copy(cT[:, m, nt * JT:(nt + 1) * JT], ps[:])
                if debug:
                    nc.sync.dma_start(dbg["d_cT"][:],
                                      cT[:].rearrange("p a b -> p (a b)"))

                rawT = RAWP.tile([128, JB, OWN], F32)
                for i in range(OWN):
                    for nt in range(NT):
                        h1 = GSM.tile([128, 2, JT], F32R, tag="h1")
                        for m in range(2):
                            nc.vector.tensor_scalar(
                                h1[:, m, :], cT[:, m, nt * JT:(nt + 1) * JT],
                                aT[:, m, i:i + 1], 0.0, op0=ALU.add, op1=ALU.max)
                        h2ps = MMP.tile([128, JT], F32, tag="mmp")
                        nc.tensor.matmul(h2ps[:], w2[:, 0, :], h1[:, 0, :],
                                         start=True, stop=False)
                        nc.tensor.matmul(h2ps[:], w2[:, 1, :], h1[:, 1, :],
                                         start=False, stop=True)
                        h2r = GSM.tile([128, JT], BF16, tag="h2r")
                        nc.scalar.activation(h2r[:], h2ps[:], AF.Relu, bias=b2[:, 0:1])
                        for jb in range(3):
                            g = nt * 3 + jb
                            nc.tensor.matmul(rawT[:, g, i:i + 1],
                                             h2r[:, jb * 128:(jb + 1) * 128], w3[:],
                                             start=True, stop=True)

                scoreT = P.tile([128, JB, OWN], F32)
                nc.scalar.activation(scoreT[:].rearrange("p a b -> p (a b)"),
                                     rawT[:].rearrange("p a b -> p (a b)"),
                                     AF.Relu, bias=b3[:, 0:1])
                scoreM = P.tile([128, JB, OWN], F32)
                nc.vector.tensor_tensor(scoreM[:], scoreT[:], maskT[:], ALU.mult)
                nc.sync.dma_start(o_score[:],
                                  scoreM[:].rearrange("p a b -> p (a b)"))

                esT = P.tile([128, JB, OWN], F32)
                nc.scalar.activation(esT[:].rearrange("p a b -> p (a b)"),
                                     scoreM[:].rearrange("p a b -> p (a b)"), AF.Exp)
                csT = P.tile([128, JB], F32)
                for g in range(JB):
                    nc.vector.tensor_reduce(csT[:, g:g + 1], esT[:, g, :],
                                            axis=AX.X, op=ALU.add)
                nc.sync.dma_start(o_colsum[:], csT[:])

    nc.compile()
    return nc


# ------------------------------------------------------------------ host glue

_NC_CACHE = {}


def _get_nc(debug=False):
    key = bool(debug)
    if key not in _NC_CACHE:
        _NC_CACHE[key] = build(debug)
    return _NC_CACHE[key]


def _in_maps(inputs):
    inp = {k: np.asarray(v) for k, v in inputs.items()}
    x = inp["x"].astype(np.float32)

    shared = {
        "wih0f": _wihT(inp["wih_l0f"], inp["bih_l0f"], inp["bhh_l0f"], EMB, 2),
        "wih0b": _wihT(inp["wih_l0b"], inp["bih_l0b"], inp["bhh_l0b"], EMB, 2),
        "wih1f": _wihT(inp["wih_l1f"], inp["bih_l1f"], inp["bhh_l1f"], 2 * H, 5),
        "wih1b": _wihT(inp["wih_l1b"], inp["bih_l1b"], inp["bhh_l1b"], 2 * H, 5),
        "whh0f": _whhT(inp["whh_l0f"]), "whh0b": _whhT(inp["whh_l0b"]),
        "whh1f": _whhT(inp["whh_l1f"]), "whh1b": _whhT(inp["whh_l1b"]),
        "w1h": _w1T(inp["w1"][:, :2 * H], inp["b1"]),
        "w1c": _w1T(inp["w1"][:, 2 * H:])[:, :4 * 2 * 128],
        "w2": np.ascontiguousarray(
            np.asarray(inp["w2"], np.float32).T.reshape(2, 128, 128)
            .transpose(1, 0, 2)).reshape(128, 256),
        "w3": np.asarray(inp["w3"], np.float32).reshape(128, 1).astype(
            ml_dtypes.bfloat16),
        "b2": np.asarray(inp["b2"], np.float32).reshape(128, 1).copy(),
        "b3": np.full((128, 1), np.float32(np.asarray(inp["b3"])[0]), np.float32),
    }

    maps = []
    for c in range(NCORES):
        xfw, xbw = _x_windows(x, c)
        # maskT[p, g, i] = valid(head = 96c+i, dep = 128g+p)
        jj = (np.arange(128)[:, None, None] + 128 * np.arange(JB)[None, :, None])
        ii = c * OWN + np.arange(OWN)[None, None, :]
        m = ((jj >= 1) & (jj != ii)).astype(np.float32)
        d = dict(shared)
        d["xf"] = xfw
        d["xb"] = xbw
        d["frzf"] = _freeze_row(c, 0, None)
        d["frzb"] = _freeze_row(c, None, NCORES - 1)
        d["maskT"] = m
        maps.append(d)
    return maps


def run_spmd(inputs, debug=False, trace=False):
    nc = _get_nc(debug=debug)
    maps = _in_maps(inputs)
    return run_bass_kernel_spmd(nc, maps, core_ids=list(range(NCORES)),
                                trace=trace)


def kernel(**inputs):
    res = run_spmd(inputs)

    score = np.zeros((N, N), np.float32)
    colsum = np.zeros((N,), np.float32)
    for c in range(NCORES):
        st = res.results[c]["scoreT"].reshape(128, JB, OWN)
        score[c * OWN:(c + 1) * OWN] = st.transpose(2, 1, 0).reshape(OWN, N)
        cs = res.results[c]["colsumT"]          # [128, JB]
        colsum += cs.T.reshape(N)

    denom = colsum - np.float32(1.0)
    tree = np.asarray(inputs["tree"])
    v1, v2 = tree[1:, 0], tree[1:, 1]
    loss = np.float32(np.mean(np.log(denom[v2]) - score[v1, v2], dtype=np.float32))
    return loss, score


# revision 20
# speedup vs baseline: 3798.3330x; 1.0250x over previous
# Trainium2 Bass kernel for nn_DependencyParser (2-layer biLSTM + edge-grid MLP).
#
# Strategy (8 NeuronCores):
#  - The n x n score grid is sharded row-wise: core c owns head rows [96c, 96c+96).
#  - The sequential biLSTM is time-chunked: core c computes LSTM states for its
#    own 96-step window after a 48-step warmup from zero state (the LSTM state
#    is contractive, so the warmup converges to the exact trajectory to ~1e-7).
#    Cores 0/7 have no real warmup; their fictitious warmup steps are "frozen"
#    by adding -40 to the i/o gate pre-activations so h,c stay ~0 (exact to fp32).
#  - After each layer, the owned windows are AllGathered so every core has the
#    full sequence for the next layer's input transform / the grid MLP.
#  - Recurrent matvec on the PE (bf16 weights); gates/states fp32; grid MLP in
#    float32r (full-rate fp32 matmul mode). The last matvec (w3) uses h2 as the
#    stationary operand so scores come out j-major (host un-transposes).

import numpy as np
import ml_dtypes

import concourse.bacc as bacc
import concourse.bass as bass
import concourse.mybir as mybir
import concourse.tile as tile
from concourse.bass import ds
from concourse.bass_utils import run_bass_kernel_spmd

AF = mybir.ActivationFunctionType
ALU = mybir.AluOpType
AX = mybir.AxisListType
F32 = mybir.dt.float32
F32R = mybir.dt.float32r
BF16 = mybir.dt.bfloat16

N = 768          # sequence length / grid size
EMB = 200        # input embedding dim
H = 256          # LSTM hidden per direction
NCORES = 8
OWN = N // NCORES   # 96 owned time steps / head rows per core
WARM = 20           # warmup steps for chunked recurrence
CHL = OWN + WARM    # chain length per core per direction (116)
FREEZE = -40.0      # gate preactivation offset that freezes the LSTM state
JB = N // 128       # 6 j-blocks of 128
NT = 2
JT = N // NT        # 384

# device gate layout: 8 col-blocks of 128 = [i0 i1 f0 f1 o0 o1 g0 g1]
_PERM_BLOCKS = [0, 1, 2, 3, 6, 7, 4, 5]   # source block (pytorch i,f,g,o order)
_FREEZE_M = (0, 1, 4, 5)                  # i and o col-blocks get FREEZE in warmup
PERM = np.concatenate([np.arange(b * 128, (b + 1) * 128) for b in _PERM_BLOCKS])


# ----------------------------------------------------------- host weight prep

def _whhT(whh):
    """[4H, H] -> [128, 2*8*128] bf16 PE lhsT blocks."""
    wp = np.asarray(whh, np.float32)[PERM]
    out = np.zeros((128, 2, 8, 128), np.float32)
    for kc in range(2):
        for m in range(8):
            out[:, kc, m, :] = wp[m * 128:(m + 1) * 128, kc * 128:(kc + 1) * 128].T
    return out.reshape(128, -1).astype(ml_dtypes.bfloat16)


def _wihT(wih, bih, bhh, din, nkc):
    """[4H, din] -> [128, nkc*8*128] fp32 lhsT; bias row at din, freeze at din+1."""
    wp = np.asarray(wih, np.float32)[PERM]
    bp = (np.asarray(bih, np.float32) + np.asarray(bhh, np.float32))[PERM]
    aug = np.zeros((1024, nkc * 128), np.float32)
    aug[:, :din] = wp
    aug[:, din] = bp
    for m in _FREEZE_M:
        aug[m * 128:(m + 1) * 128, din + 1] = FREEZE
    out = np.zeros((128, nkc, 8, 128), np.float32)
    for kc in range(nkc):
        for m in range(8):
            out[:, kc, m, :] = aug[m * 128:(m + 1) * 128, kc * 128:(kc + 1) * 128].T
    return out.reshape(128, -1)


def _x_windows(x, core):
    """Per-core augmented input windows, fp32 [128, 2, CHL] for f and b dirs."""
    x = np.asarray(x, np.float32)
    outs = []
    for t0 in (core * OWN - WARM, core * OWN):
        w = np.zeros((256, CHL), np.float32)
        for tt in range(CHL):
            t = t0 + tt
            if 0 <= t < N:
                w[:EMB, tt] = x[t]
                w[EMB, tt] = 1.0
            else:
                w[EMB, tt] = 1.0
                w[EMB + 1, tt] = 1.0
        outs.append(w.reshape(2, 128, CHL).transpose(1, 0, 2).copy())
    return outs  # [xf, xb]


def _w1T(w1half, b1=None):
    """[256, 512] -> [128, 5*2*128] fp32 lhsT blocks (+bias chunk rows if b1)."""
    w = np.asarray(w1half, np.float32)
    out = np.zeros((128, 5, 2, 128), np.float32)
    for kc in range(4):
        for m in range(2):
            out[:, kc, m, :] = w[m * 128:(m + 1) * 128, kc * 128:(kc + 1) * 128].T
    if b1 is not None:
        b = np.asarray(b1, np.float32)
        for m in range(2):
            out[0, 4, m, :] = b[m * 128:(m + 1) * 128]
    return out.reshape(128, -1)


def _freeze_row(core, lo_frozen_core, hi_frozen_core):
    """[1, CHL] warmup-freeze indicator rows for phase-B aug chunk."""
    r = np.zeros((1, CHL), np.float32)
    if core == lo_frozen_core:
        r[0, :WARM] = 1.0
    if core == hi_frozen_core:
        r[0, CHL - WARM:] = 1.0
    return r


# --------------------------------------------------------------- bass program

def build(debug=False, sim_nocc=False):
    """sim_nocc=True replaces collectives with equivalent local DMA traffic so
    the single-core TimelineSim cost model can run the program (timing only)."""
    nc = bacc.Bacc("TRN2", target_bir_lowering=False, debug=False,
                   num_devices=NCORES)

    def din(name, shape, dt=F32):
        return nc.dram_tensor(name, shape, dt, kind="ExternalInput")

    i_xf = din("xf", [128, 2, CHL])
    i_xb = din("xb", [128, 2, CHL])
    i_wih0f = din("wih0f", [128, 2 * 8 * 128])
    i_wih0b = din("wih0b", [128, 2 * 8 * 128])
    i_wih1f = din("wih1f", [128, 5 * 8 * 128])
    i_wih1b = din("wih1b", [128, 5 * 8 * 128])
    i_whh = {d: din(f"whh{d}", [128, 2 * 8 * 128], BF16)
             for d in ("0f", "0b", "1f", "1b")}
    i_frzf = din("frzf", [1, CHL])
    i_frzb = din("frzb", [1, CHL])
    i_w1h = din("w1h", [128, 5 * 2 * 128], F32R)
    i_w1c = din("w1c", [128, 4 * 2 * 128], F32R)
    i_w2 = din("w2", [128, 2 * 128], F32R)
    i_w3 = din("w3", [128, 1], BF16)
    i_b2 = din("b2", [128, 1])
    i_b3 = din("b3", [128, 1])
    i_mask = din("maskT", [128, JB, OWN])

    o_score = nc.dram_tensor("scoreT", [128, JB * OWN], F32, kind="ExternalOutput")
    o_colsum = nc.dram_tensor("colsumT", [128, JB], F32, kind="ExternalOutput")
    dbg = {}
    if debug:
        for nm, shp, dt in (("d_xw0f", [128, 8 * CHL], F32),
                            ("d_h0", [128, 4 * (N + 2 * WARM)], F32),
                            ("d_xw1f", [128, 8 * CHL], F32),
                            ("d_emb", [128, 4 * N], F32R),
                            ("d_aT", [128, 2 * OWN], F32),
                            ("d_cT", [128, 2 * N], F32)):
            dbg[nm] = nc.dram_tensor(nm, shp, dt, kind="ExternalOutput")

    HPAD = N + 2 * WARM

    with tile.TileContext(nc) as tc:
        with (
            tc.tile_pool(name="persist", bufs=1) as P,
            tc.tile_pool(name="mmp", bufs=2, space="PSUM") as MMP,
            tc.tile_pool(name="dram", bufs=1, space="DRAM") as DR,
        ):
            # ---- load static inputs to SBUF
            xf = P.tile([128, 2, CHL], F32)
            nc.sync.dma_start(xf[:], i_xf[:])
            xb = P.tile([128, 2, CHL], F32)
            nc.sync.dma_start(xb[:], i_xb[:])
            wih0 = {}
            for d, inp in (("f", i_wih0f), ("b", i_wih0b)):
                t = P.tile([128, 2, 8, 128], F32, tag=f"wih0{d}")
                nc.sync.dma_start(t[:], inp[:].rearrange("p (a b c) -> p a b c", a=2, b=8))
                wih0[d] = t
            wih1 = {}
            for d, inp in (("f", i_wih1f), ("b", i_wih1b)):
                t = P.tile([128, 5, 8, 128], F32, tag=f"wih1{d}")
                nc.sync.dma_start(t[:], inp[:].rearrange("p (a b c) -> p a b c", a=5, b=8))
                wih1[d] = t
            whh = {}
            for d in ("0f", "0b", "1f", "1b"):
                t = P.tile([128, 2, 8, 128], BF16, tag=f"whh{d}")
                nc.sync.dma_start(t[:], i_whh[d][:].rearrange("p (a b c) -> p a b c", a=2, b=8))
                whh[d] = t
            w1h = P.tile([128, 5, 2, 128], F32R)
            nc.sync.dma_start(w1h[:], i_w1h[:].rearrange("p (a b c) -> p a b c", a=5, b=2))
            w1c = P.tile([128, 4, 2, 128], F32R)
            nc.sync.dma_start(w1c[:], i_w1c[:].rearrange("p (a b c) -> p a b c", a=4, b=2))
            w2 = P.tile([128, 2, 128], F32R)
            nc.sync.dma_start(w2[:], i_w2[:].rearrange("p (a b) -> p a b", a=2))
            w3 = P.tile([128, 1], BF16)
            nc.sync.dma_start(w3[:], i_w3[:])
            b2 = P.tile([128, 1], F32)
            nc.sync.dma_start(b2[:], i_b2[:])
            b3 = P.tile([128, 1], F32)
            nc.sync.dma_start(b3[:], i_b3[:])
            maskT = P.tile([128, JB, OWN], F32)
            nc.sync.dma_start(maskT[:], i_mask[:])

            pid = nc.vector.partition_id()

            def xw_from(lhsT, rhs, nkc, xw_out):
                """xw_out[128, 8, CHL] fp32 = sum_kc lhsT[:,kc,m,:].T @ rhs[:,kc,:]"""
                for m in range(8):
                    ps = MMP.tile([128, CHL], F32, tag="mmp")
                    for kc in range(nkc):
                        nc.tensor.matmul(ps[:], lhsT[:, kc, m, :], rhs[:, kc, :],
                                         start=(kc == 0), stop=(kc == nkc - 1))
                    nc.vector.tensor_copy(xw_out[:, m, :], ps[:])

            xw_f = P.tile([128, 8, CHL], F32)
            xw_b = P.tile([128, 8, CHL], F32)
            h_ownT = P.tile([128, 4, OWN], F32)
            emb_ownT = P.tile([128, 4, OWN], F32R)
            h0_pad = P.tile([128, 4, HPAD], F32)

            with (
                tc.tile_pool(name="small", bufs=3) as SM,
                tc.tile_pool(name="hp", bufs=2) as HP,
                tc.tile_pool(name="gpsf", bufs=2, space="PSUM") as GPSF,
                tc.tile_pool(name="gpsb", bufs=2, space="PSUM") as GPSB,
            ):
                def chain(name, whhT, xw_sb, gpool, gtag, storeT, store_base, rev):
                    """One LSTM chain of CHL steps; owned h written to storeT."""
                    TC = P.tile([128, 4], F32, tag=f"TC_{name}")  # [tg0 tg1 c0 c1]
                    nc.vector.memset(TC[:, 2:4], 0.0)
                    h_bf = HP.tile([128, 2], BF16, tag=f"h_{name}")
                    nc.vector.memset(h_bf[:], 0.0)
                    for ss in range(CHL):
                        col = (CHL - 1 - ss) if rev else ss
                        gps = gpool.tile([128, 8], F32, tag=gtag)
                        for m in range(8):
                            nc.tensor.matmul(gps[:, m:m + 1], whhT[:, 0, m, :],
                                             h_bf[:, 0:1], start=True, stop=False)
                            nc.tensor.matmul(gps[:, m:m + 1], whhT[:, 1, m, :],
                                             h_bf[:, 1:2], start=False, stop=True)
                        gates = SM.tile([128, 8], F32, tag=f"g_{name}")
                        nc.vector.tensor_tensor(gates[:], gps[:], xw_sb[:, :, col],
                                                ALU.add)
                        S = SM.tile([128, 6], F32, tag=f"S_{name}")
                        nc.scalar.activation(S[:], gates[:, 0:6], AF.Sigmoid)
                        nc.scalar.activation(TC[:, 0:2], gates[:, 6:8], AF.Tanh)
                        Pp = SM.tile([128, 4], F32, tag=f"P_{name}")
                        nc.vector.tensor_tensor(Pp[:], S[:, 0:4], TC[:], ALU.mult)
                        nc.vector.tensor_tensor(TC[:, 2:4], Pp[:, 0:2], Pp[:, 2:4],
                                                ALU.add)
                        T2 = SM.tile([128, 2], F32, tag=f"T2_{name}")
                        nc.scalar.activation(T2[:], TC[:, 2:4], AF.Tanh)
                        h_bf = HP.tile([128, 2], BF16, tag=f"h_{name}")
                        nc.vector.tensor_tensor(h_bf[:], S[:, 4:6], T2[:], ALU.mult)
                        if ss >= WARM:
                            idx = (ss - WARM) if not rev else (CHL - 1 - ss)
                            nc.vector.tensor_tensor(
                                storeT[:, store_base:store_base + 2, idx:idx + 1],
                                S[:, 4:6], T2[:], ALU.mult)

                # ========== phase A: layer 0 ==========
                xw_from(wih0["f"], xf, 2, xw_f)
                xw_from(wih0["b"], xb, 2, xw_b)
                if debug:
                    nc.sync.dma_start(dbg["d_xw0f"][:],
                                      xw_f[:].rearrange("p a b -> p (a b)"))

                chain("a_f", whh["0f"], xw_f, GPSF, "gf", h_ownT, 0, rev=False)
                chain("a_b", whh["0b"], xw_b, GPSB, "gb", h_ownT, 2, rev=True)

                cc1_in = DR.tile([4, 128, OWN], F32)
                cc1_out = DR.tile([NCORES, 4, 128, OWN], F32,
                                  addr_space="Local" if sim_nocc else "Shared")
                nc.sync.dma_start(cc1_in[:].rearrange("c p k -> p c k"), h_ownT[:])
                if sim_nocc:
                    for b in range(NCORES):
                        nc.sync.dma_start(cc1_out[b], cc1_in[:])
                else:
                    nc.gpsimd.collective_compute(
                        "AllGather", ALU.bypass,
                        replica_groups=[list(range(NCORES))],
                        ins=[cc1_in.opt()], outs=[cc1_out.opt()])

                nc.vector.memset(h0_pad[:, :, 0:WARM], 0.0)
                nc.vector.memset(h0_pad[:, :, WARM + N:], 0.0)
                for ch in range(4):
                    nc.sync.dma_start(
                        h0_pad[:, ch, WARM:WARM + N].rearrange("p (b k) -> p b k", k=OWN),
                        cc1_out[:, ch, :, :].rearrange("b p k -> p b k"))
                if debug:
                    nc.sync.dma_start(dbg["d_h0"][:],
                                      h0_pad[:].rearrange("p a b -> p (a b)"))

                # ========== phase B: layer 1 ==========
                h0w = {}
                for d, off, frz in (("f", pid * OWN, i_frzf),
                                    ("b", pid * OWN + WARM, i_frzb)):
                    hw = P.tile([128, 5, CHL], F32, tag=f"h0w_{d}")
                    nc.vector.tensor_copy(hw[:, 0:4, :], h0_pad[:, :, ds(off, CHL)])
                    nc.vector.memset(hw[:, 4, :], 0.0)
                    nc.vector.memset(hw[0:1, 4, :], 1.0)
                    nc.sync.dma_start(hw[1:2, 4, :], frz[:])
                    h0w[d] = hw

                xw_from(wih1["f"], h0w["f"], 5, xw_f)
                xw_from(wih1["b"], h0w["b"], 5, xw_b)
                if debug:
                    nc.sync.dma_start(dbg["d_xw1f"][:],
                                      xw_f[:].rearrange("p a b -> p (a b)"))

                chain("b_f", whh["1f"], xw_f, GPSF, "gf", emb_ownT, 0, rev=False)
                chain("b_b", whh["1b"], xw_b, GPSB, "gb", emb_ownT, 2, rev=True)

            # ---- AllGather embeddings
            cc2_in = DR.tile([4, 128, OWN], F32R)
            cc2_out = DR.tile([NCORES, 4, 128, OWN], F32R,
                              addr_space="Local" if sim_nocc else "Shared")
            nc.sync.dma_start(cc2_in[:].rearrange("c p k -> p c k"), emb_ownT[:])
            if sim_nocc:
                for b in range(NCORES):
                    nc.sync.dma_start(cc2_out[b], cc2_in[:])
            else:
                nc.gpsimd.collective_compute(
                    "AllGather", ALU.bypass,
                    replica_groups=[list(range(NCORES))],
                    ins=[cc2_in.opt()], outs=[cc2_out.opt()])

            embT = P.tile([128, 4, N], F32R)
            for ch in range(4):
                nc.sync.dma_start(
                    embT[:, ch, :].rearrange("p (b k) -> p b k", k=OWN),
                    cc2_out[:, ch, :, :].rearrange("b p k -> p b k"))
            if debug:
                nc.sync.dma_start(dbg["d_emb"][:],
                                  embT[:].rearrange("p a b -> p (a b)"))

            # ========== grid MLP (scores come out j-major) ==========
            with (
                tc.tile_pool(name="gridsm", bufs=3) as GSM,
                tc.tile_pool(name="rawp", bufs=1, space="PSUM") as RAWP,
            ):
                ones1f = P.tile([1, OWN], F32)
                nc.vector.memset(ones1f[:], 1.0)
                ones1 = P.tile([1, OWN], F32R)
                nc.vector.tensor_copy(ones1[:], ones1f[:])

                aT = P.tile([128, 2, OWN], F32)
                for m in range(2):
                    ps = MMP.tile([128, OWN], F32, tag="mmp")
                    for kc in range(4):
                        nc.tensor.matmul(ps[:], w1h[:, kc, m, :], emb_ownT[:, kc, :],
                                         start=(kc == 0), stop=False)
                    nc.tensor.matmul(ps[:], w1h[0:1, 4, m, :], ones1[:],
                                     start=False, stop=True)
                    nc.vector.tensor_copy(aT[:, m, :], ps[:])
                if debug:
                    nc.sync.dma_start(dbg["d_aT"][:],
                                      aT[:].rearrange("p a b -> p (a b)"))

                cT = P.tile([128, 2, N], F32)
                for m in range(2):
                    for nt in range(NT):
                        ps = MMP.tile([128, JT], F32, tag="mmp")
                        for kc in range(4):
                            nc.tensor.matmul(ps[:], w1c[:, kc, m, :],
                                             embT[:, kc, nt * JT:(nt + 1) * JT],
                                             start=(kc == 0), stop=(kc == 3))
                        nc.vector.tensor_copy(cT[:, m, nt * JT:(nt + 1) * JT], ps[:])
                if debug:
                    nc.sync.dma_start(dbg["d_cT"][:],
                                      cT[:].rearrange("p a b -> p (a b)"))

                rawT = RAWP.tile([128, JB, OWN], F32)
                for i in range(OWN):
                    for nt in range(NT):
                        h1 = GSM.tile([128, 2, JT], F32R, tag="h1")
                        for m in range(2):
                            nc.vector.tensor_scalar(
                                h1[:, m, :], cT[:, m, nt * JT:(nt + 1) * JT],
                                aT[:, m, i:i + 1], 0.0, op0=ALU.add, op1=ALU.max)
                        h2ps = MMP.tile([128, JT], F32, tag="mmp")
                        nc.tensor.matmul(h2ps[:], w2[:, 0, :], h1[:, 0, :],
                                         start=True, stop=False)
                        nc.tensor.matmul(h2ps[:], w2[:, 1, :], h1[:, 1, :],
                                         start=False, stop=True)
                        h2r = GSM.tile([128, JT], BF16, tag="h2r")
                        nc.scalar.activation(h2r[:], h2ps[:], AF.Relu, bias=b2[:, 0:1])
                        for jb in range(3):
                            g = nt * 3 + jb
                            nc.tensor.matmul(rawT[:, g, i:i + 1],
                                             h2r[:, jb * 128:(jb + 1) * 128], w3[:],
                                             start=True, stop=True)

                scoreT = P.tile([128, JB, OWN], F32)
                nc.scalar.activation(scoreT[:].rearrange("p a b -> p (a b)"),
                                     rawT[:].rearrange("p a b -> p (a b)"),
                                     AF.Relu, bias=b3[:, 0:1])
                scoreM = P.tile([128, JB, OWN], F32)
                nc.vector.tensor_tensor(scoreM[:], scoreT[:], maskT[:], ALU.mult)
                nc.sync.dma_start(o_score[:],
                                  scoreM[:].rearrange("p a b -> p (a b)"))

                esT = P.tile([128, JB, OWN], F32)
                nc.scalar.activation(esT[:].rearrange("p a b -> p (a b)"),
                                     scoreM[:].rearrange("p a b -> p (a b)"), AF.Exp)
                csT = P.tile([128, JB], F32)
                for g in range(JB):
                    nc.vector.tensor_reduce(csT[:, g:g + 1], esT[:, g, :],
                                            axis=AX.X, op=ALU.add)
                nc.sync.dma_start(o_colsum[:], csT[:])

    nc.compile()
    return nc


# ------------------------------------------------------------------ host glue

_NC_CACHE = {}


def _get_nc(debug=False):
    key = bool(debug)
    if key not in _NC_CACHE:
        _NC_CACHE[key] = build(debug)
    return _NC_CACHE[key]


def _in_maps(inputs):
    inp = {k: np.asarray(v) for k, v in inputs.items()}
    x = inp["x"].astype(np.float32)

    shared = {
        "wih0f": _wihT(inp["wih_l0f"], inp["bih_l0f"], inp["bhh_l0f"], EMB, 2),
        "wih0b": _wihT(inp["wih_l0b"], inp["bih_l0b"], inp["bhh_l0b"], EMB, 2),
        "wih1f": _wihT(inp["wih_l1f"], inp["bih_l1f"], inp["bhh_l1f"], 2 * H, 5),
        "wih1b": _wihT(inp["wih_l1b"], inp["bih_l1b"], inp["bhh_l1b"], 2 * H, 5),
        "whh0f": _whhT(inp["whh_l0f"]), "whh0b": _whhT(inp["whh_l0b"]),
        "whh1f": _whhT(inp["whh_l1f"]), "whh1b": _whhT(inp["whh_l1b"]),
        "w1h": _w1T(inp["w1"][:, :2 * H], inp["b1"]),
        "w1c": _w1T(inp["w1"][:, 2 * H:])[:, :4 * 2 * 128],
        "w2": np.ascontiguousarray(
            np.asarray(inp["w2"], np.float32).T.reshape(2, 128, 128)
            .transpose(1, 0, 2)).reshape(128, 256),
        "w3": np.asarray(inp["w3"], np.float32).reshape(128, 1).astype(
            ml_dtypes.bfloat16),
        "b2": np.asarray(inp["b2"], np.float32).reshape(128, 1).copy(),
        "b3": np.full((128, 1), np.float32(np.asarray(inp["b3"])[0]), np.float32),
    }

    maps = []
    for c in range(NCORES):
        xfw, xbw = _x_windows(x, c)
        # maskT[p, g, i] = valid(head = 96c+i, dep = 128g+p)
        jj = (np.arange(128)[:, None, None] + 128 * np.arange(JB)[None, :, None])
        ii = c * OWN + np.arange(OWN)[None, None, :]
        m = ((jj >= 1) & (jj != ii)).astype(np.float32)
        d = dict(shared)
        d["xf"] = xfw
        d["xb"] = xbw
        d["frzf"] = _freeze_row(c, 0, None)
        d["frzb"] = _freeze_row(c, None, NCORES - 1)
        d["maskT"] = m
        maps.append(d)
    return maps


def run_spmd(inputs, debug=False, trace=False):
    nc = _get_nc(debug=debug)
    maps = _in_maps(inputs)
    return run_bass_kernel_spmd(nc, maps, core_ids=list(range(NCORES)),
                                trace=trace)


def kernel(**inputs):
    res = run_spmd(inputs)

    score = np.zeros((N, N), np.float32)
    colsum = np.zeros((N,), np.float32)
    for c in range(NCORES):
        st = res.results[c]["scoreT"].reshape(128, JB, OWN)
        score[c * OWN:(c + 1) * OWN] = st.transpose(2, 1, 0).reshape(OWN, N)
        cs = res.results[c]["colsumT"]          # [128, JB]
        colsum += cs.T.reshape(N)

    denom = colsum - np.float32(1.0)
    tree = np.asarray(inputs["tree"])
    v1, v2 = tree[1:, 0], tree[1:, 1]
    loss = np.float32(np.mean(np.log(denom[v2]) - score[v1, v2], dtype=np.float32))
    return loss, score
